# revision 29
# baseline (speedup 1.0000x reference)
"""Fused multi-head attention (qkv + RoPE + softmax + proj) for TRN2, 8 cores.

Sharding: core c -> batch b=c//2, head group hg=c%2 (8 of 16 heads).
Data-parallel over B (4), 2-way tensor-parallel over heads.
Host unshard: out[b] = partial[2b] + partial[2b+1] + b_proj.

v5 (411us -> ~381us): PE-first schedule.  Profiling showed the tensor
engine is the bottleneck (341us busy), not the exp stream (277us):
  1. RoPE rotate-half runs as a DVE stream_shuffle instead of a PE
     matmul: head-dim channels are permuted host-side so each rope pair
     (d, d+32) lands in one 32-partition quadrant; the sign is folded
     into the sin table.  Kills 32 PE matmuls + 32 PSUM casts + the p2t
     constant + one PSUM bank.
  2. Every producer unit (q/k/v/proj) is split into single-matmul steps
     fed by a per-ki pacer whose budget LEVELS the PE load at ~4.5
     units/ki: heavy early (the v/k/q front-load the first block needs),
     1/ki through the ACT-bound middle, 2/ki in the proj-heavy back.
  3. AV matmuls drain from an elastic FIFO (e-ring EPOOL=20).
     Emission-order rule (engine queues are in-order, a cycle
     deadlocks): AV(j) must be emitted before exp(j+EPOOL) reuses its
     e-slot, which sets every v-unit's emission deadline; asserts
     enforce them at build time.
  4. proj is split: hp0-2 partial sums run during the ACT-bound middle
     (gated on the hp2 block of their chunk) and park in f32 bitcast
     views of the dead xs memory; the hp3 final step adds partial+psum.
     Slot reuse order [pc0 pc1 fc0 pc2 fc1 pc3 fc2] keeps the in-order
     DVE queue deadlock-free.
  5. Batched DMA (~30 issues vs 133; the sync engine takes ~600ns per
     issue), ordered so the first k-unit starts ~9us in; x chunk 0 is
     split into 4 pieces so matmuls chase the DMA.
  6. Tail: lag taper, latency-optimized last-block normalize, and the 8
     final proj matmuls go to borrowed psum slots before their adds.
Steady state: ACT-bound middle at ~1078ns/ki, PE-bound front/back.
bf16 everywhere (incl cos/sin tables; numpy-sim rel err 1.1e-2 vs the
2e-2 gate); rope arithmetic f32 in PSUM.
"""

import sys

if "/opt/trn_rl_repo" not in sys.path:
    sys.path.insert(0, "/opt/trn_rl_repo")

import numpy as np
from contextlib import ExitStack

B, N, C, H, D = 4, 2048, 1024, 16, 64
NCORES = 8
P = 128
DH = 512          # per-core head channels (8 heads x 64)
CT = C // P       # 8 contraction tiles for qkv
DHT = DH // P     # 4 partition tiles of qT/kT/aoT (= head pairs)
NCH = N // 512    # 4 n chunks of 512
KT = N // P       # 16 key tiles
NBLK = DHT * NCH  # 16 attention blocks
EPOOL = 20        # e-tile ring; AV lag hard bound = EPOOL - 2
VSTR = 72         # per-head stride in the packed v tile (64 d + ones + pad)

# stream_shuffle mask: within each 32-partition quadrant swap the 16-halves
SHUF_MASK = list(range(16, 32)) + list(range(16))

_CACHE = {}


def _soft_lag(g):
    if g >= 246:
        return 1
    if g >= 240:
        return 2
    if g >= 224:
        return 3
    if g >= 192:
        return 6
    return 15


def _budget(g):
    if g < 24:
        return 6
    if g < 36:
        return 4
    if g < 72:
        return 2
    if g < 160:
        return 1
    return 2


def _emit(nc, tc, mybir, bass, tile):
    F32 = mybir.dt.float32
    BF16 = mybir.dt.bfloat16
    Exp = mybir.ActivationFunctionType.Exp

    xT = nc.dram_tensor("xT", [C, N], BF16, kind="ExternalInput").ap()
    wq = nc.dram_tensor("wq", [C, DH], BF16, kind="ExternalInput").ap()
    wk = nc.dram_tensor("wk", [C, DH], BF16, kind="ExternalInput").ap()
    wv = nc.dram_tensor("wv", [C, DH], BF16, kind="ExternalInput").ap()
    wp = nc.dram_tensor("wp", [DH, C], BF16, kind="ExternalInput").ap()
    cosp = nc.dram_tensor("cosp", [P, N], BF16, kind="ExternalInput").ap()
    ssin = nc.dram_tensor("ssin", [P, N], BF16, kind="ExternalInput").ap()
    onesd = nc.dram_tensor("onesd", [P, P], BF16, kind="ExternalInput").ap()
    out = nc.dram_tensor("out", [N, C], BF16, kind="ExternalOutput").ap()

    ctx = ExitStack()
    with ctx:
        consts = ctx.enter_context(tc.tile_pool(name="consts", bufs=1))
        persist = ctx.enter_context(tc.tile_pool(name="persist", bufs=1))

        cos_c = [consts.tile([P, 512], BF16, tag=f"cos{c}", name=f"cos{c}")
                 for c in range(NCH)]
        sin_c = [consts.tile([P, 512], BF16, tag=f"sin{c}", name=f"sin{c}")
                 for c in range(NCH)]

        qTc = [[persist.tile([P, 512], BF16, tag=f"qT{t}_{c}",
                             name=f"qT{t}_{c}")
                for c in range(NCH)] for t in range(DHT)]
        kTc = [[persist.tile([P, 512], BF16, tag=f"kT{t}_{c}",
                             name=f"kT{t}_{c}")
                for c in range(NCH)] for t in range(DHT)]
        aoTc = [[persist.tile([P, 512], BF16, tag=f"aoT{t}_{c}",
                              name=f"aoT{t}_{c}")
                 for c in range(NCH)] for t in range(DHT)]
        # one packed tile for all v key tiles: [128 keys, 16 ki x 8 h x 72]
        vS = persist.tile([P, KT * 8 * VSTR], BF16, tag="vS", name="vS")
        wp_sb = [persist.tile([P, C], BF16, tag=f"wp{i}", name=f"wp{i}")
                 for i in range(DHT)]
        # per head-pair qkv weights: [128 in-ch, 8 kc x 128 out-ch]
        wq_sb = [persist.tile([P, CT * P], BF16, tag=f"wq{t}", name=f"wq{t}")
                 for t in range(DHT)]
        wk_sb = [persist.tile([P, CT * P], BF16, tag=f"wk{t}", name=f"wk{t}")
                 for t in range(DHT)]
        wv_sb = persist.tile([P, CT * DH], BF16, tag="wv", name="wv")
        # x chunks: [128 in-ch, 8 kc x 512 n]
        xs = [persist.tile([P, CT * 512], BF16, tag=f"x{c}", name=f"x{c}")
              for c in range(NCH)]

        upool = ctx.enter_context(tc.tile_pool(name="upool", bufs=2,
                                               space="PSUM"))
        spool = ctx.enter_context(tc.tile_pool(name="spool", bufs=1,
                                               space="PSUM"))
        opool = ctx.enter_context(tc.tile_pool(name="opool", bufs=1,
                                               space="PSUM"))
        epool = ctx.enter_context(tc.tile_pool(name="epool", bufs=EPOOL))
        shufp = ctx.enter_context(tc.tile_pool(name="shufp", bufs=1))
        ttp = ctx.enter_context(tc.tile_pool(name="ttp", bufs=1))
        finp = ctx.enter_context(tc.tile_pool(name="finp", bufs=2))
        obp = ctx.enter_context(tc.tile_pool(name="obp", bufs=2))

        # ---------------- DMA (batched, first-need-first) ----------------
        def dma_x(c, kc0, kc1):
            dst = bass.AP(tensor=xs[c].tensor, offset=kc0 * 512,
                          ap=[list(xs[c].ap[0]), [512, kc1 - kc0], [1, 512]])
            src = bass.AP(tensor=xT.tensor, offset=kc0 * P * N + c * 512,
                          ap=[[N, P], [P * N, kc1 - kc0], [1, 512]])
            nc.sync.dma_start(dst, src)

        def dma_w(dst_tile, w_ap, t):
            dst = bass.AP(tensor=dst_tile.tensor, offset=0,
                          ap=[list(dst_tile.ap[0]), [P, CT], [1, P]])
            src = bass.AP(tensor=w_ap.tensor, offset=t * P,
                          ap=[[DH, P], [P * DH, CT], [1, P]])
            nc.sync.dma_start(dst, src)

        # first k-unit's tiles land first, in small pieces so matmuls can
        # start while the rest streams; x chunks 1-3 (big, ring-clogging)
        # go after everything the front needs
        dma_x(0, 0, 1)
        dma_w(wk_sb[0], wk, 0)
        dma_x(0, 1, 2)
        dma_x(0, 2, 4)
        dma_w(wq_sb[0], wq, 0)
        dma_x(0, 4, 6)
        dma_x(0, 6, 8)
        dma_x(1, 0, 4)
        dma_x(1, 4, 8)
        nc.sync.dma_start(cos_c[0], cosp[:, 0:512])
        nc.sync.dma_start(sin_c[0], ssin[:, 0:512])
        wv_dst = bass.AP(tensor=wv_sb.tensor, offset=0,
                         ap=[list(wv_sb.ap[0]), [DH, CT], [1, DH]])
        wv_src = bass.AP(tensor=wv.tensor, offset=0,
                         ap=[[DH, P], [P * DH, CT], [1, DH]])
        nc.sync.dma_start(wv_dst, wv_src)
        dma_x(2, 0, 4)
        dma_x(2, 4, 8)
        nc.sync.dma_start(cos_c[1], cosp[:, 512:1024])
        nc.sync.dma_start(sin_c[1], ssin[:, 512:1024])
        dma_x(3, 0, 4)
        dma_x(3, 4, 8)
        for c in range(2, NCH):
            nc.sync.dma_start(cos_c[c], cosp[:, c * 512:(c + 1) * 512])
            nc.sync.dma_start(sin_c[c], ssin[:, c * 512:(c + 1) * 512])
        # ones columns into vS: offset 64 within each 72-stride head block
        ones_dst = bass.AP(tensor=vS.tensor, offset=64,
                           ap=[list(vS.ap[0]), [8 * VSTR, KT], [VSTR, 8]])
        ones_src = bass.AP(tensor=onesd.tensor, offset=0,
                           ap=[[P, P], [8, KT], [1, 8]])
        nc.sync.dma_start(ones_dst, ones_src)
        for t in range(1, DHT):
            dma_w(wk_sb[t], wk, t)
            dma_w(wq_sb[t], wq, t)
        for i in range(DHT):
            nc.sync.dma_start(wp_sb[i], wp[i * P:(i + 1) * P, :])

        # ---------------- emission-order tracking ----------------
        v_emitted = [False] * KT
        qk_emitted = set()   # ("q"|"k", t, c)

        # ---------------- unit step factories ----------------
        def qk_unit_steps(kind, w_sb, dst, t, c):
            hold = {}

            def step(kc):
                def f():
                    if kc == 0:
                        hold["ps"] = upool.tile([P, 512], F32, tag="ups", name="ups")
                    ps = hold["ps"]
                    nc.tensor.matmul(
                        ps, w_sb[t][:, kc * P:(kc + 1) * P],
                        xs[c][:, kc * 512:(kc + 1) * 512],
                        start=(kc == 0), stop=(kc == CT - 1))
                    if kc == CT - 1:
                        shuf = shufp.tile([P, 512], F32, tag="shuf", name="shuf")
                        nc.vector.stream_shuffle(shuf, ps, SHUF_MASK)
                        t1 = ttp.tile([P, 512], F32, tag="t1", name="t1")
                        nc.vector.tensor_mul(t1, ps, cos_c[c])
                        t2 = ttp.tile([P, 512], F32, tag="t2", name="t2")
                        nc.vector.tensor_mul(t2, shuf, sin_c[c])
                        nc.vector.tensor_add(dst[t][c], t1, t2)
                        qk_emitted.add((kind, t, c))
                return f
            return [step(kc) for kc in range(CT)]

        def v_unit_steps(c, nt4):
            i = c * 4 + nt4
            hold = {}

            def step(kc):
                def f():
                    if kc == 0:
                        hold["ps"] = upool.tile([P, 512], F32, tag="ups", name="ups")
                    ps = hold["ps"]
                    nc.tensor.matmul(
                        ps,
                        xs[c][:, kc * 512 + nt4 * P:kc * 512 + (nt4 + 1) * P],
                        wv_sb[:, kc * DH:(kc + 1) * DH],
                        start=(kc == 0), stop=(kc == CT - 1))
                    if kc == CT - 1:
                        v_view = bass.AP(
                            tensor=vS.tensor, offset=i * 8 * VSTR,
                            ap=[list(vS.ap[0]), [VSTR, 8], [1, 64]])
                        nc.vector.tensor_copy(
                            v_view, ps.rearrange("p (h d) -> p h d", h=8))
                        v_emitted[i] = True
                return f
            return [step(kc) for kc in range(CT)]

        # proj is split: hp0-2 partial sums run early (gated on the hp2
        # block of their chunk) and park in f32 views of the dead xs
        # memory; the hp3 final step adds partial + psum -> out.
        def pp_view(nt, fc):
            j = nt // 4
            slot = (j % 2) * 8 + (nt % 4) * 2 + fc
            base = xs[slot // 4][:, 0:CT * 512].bitcast(F32)
            return base[:, (slot % 4) * 512:(slot % 4 + 1) * 512]

        def proj_partial_steps(nt, fc):
            hold = {}

            def step(ct):
                def f():
                    if ct == 0:
                        hold["ps"] = upool.tile([P, 512], F32, tag="ups", name="ups")
                    ps = hold["ps"]
                    nc.tensor.matmul(
                        ps,
                        aoTc[ct][nt // 4][:, (nt % 4) * P:(nt % 4 + 1) * P],
                        wp_sb[ct][:, fc * 512:(fc + 1) * 512],
                        start=(ct == 0), stop=(ct == DHT - 2))
                    if ct == DHT - 2:
                        nc.vector.tensor_copy(pp_view(nt, fc), ps)
                return f
            return [step(ct) for ct in range(DHT - 1)]

        def proj_final_step(nt, fc):
            def f():
                ct = DHT - 1
                ps = upool.tile([P, 512], F32, tag="ups", name="ups")
                nc.tensor.matmul(
                    ps,
                    aoTc[ct][nt // 4][:, (nt % 4) * P:(nt % 4 + 1) * P],
                    wp_sb[ct][:, fc * 512:(fc + 1) * 512],
                    start=True, stop=True)
                ob = obp.tile([P, 512], BF16, tag="ob", name="ob")
                nc.vector.tensor_add(ob, pp_view(nt, fc), ps)
                nc.sync.dma_start(
                    out[nt * P:(nt + 1) * P, fc * 512:(fc + 1) * 512], ob)
            return [f]

        # ---------------- attention primitives ----------------
        pend = []          # FIFO of (blk, ki, e_tile)
        cur = {"blk": None, "o": None}
        finished = set()
        finished_at = {}
        gnow = [0]

        def finish_block(blk, o):
            hp, qc = blk // NCH, blk % NCH
            for par in range(2):
                pb = par * 64
                stage = finp.tile([P, 512], F32, tag="stg", name="stg")
                nc.vector.tensor_copy(stage[0:64, :], o[par][0:64, :])
                dd = finp.tile([P, 512], F32, tag="dd", name="dd")
                nc.vector.tensor_copy(dd[0:1, :], o[par][64:65, :])
                r = finp.tile([P, 512], F32, tag="r", name="r")
                nc.vector.reciprocal_approx_fast(r[0:1, :], dd[0:1, :])
                rb = finp.tile([P, 512], F32, tag="rb", name="rb")
                nc.gpsimd.partition_broadcast(
                    rb[0:64, :], r[0:1, :], channels=64)
                nc.vector.tensor_mul(
                    aoTc[hp][qc][pb:pb + 64, :], stage[0:64, :],
                    rb[0:64, :])
            finished.add(blk)
            finished_at[blk] = gnow[0]

        def pop_av():
            blk, ki, e = pend.pop(0)
            assert v_emitted[ki], f"v[{ki}] not emitted before AV pop"
            if blk != cur["blk"]:
                if cur["blk"] is not None:
                    finish_block(cur["blk"], cur["o"])
                cur["blk"] = blk
                cur["o"] = {par: opool.tile([P, 512], F32, tag=f"o{par}",
                                            name=f"o{par}")
                            for par in range(2)}
            for par in range(2):
                h = (blk // NCH) * 2 + par
                off = ki * 8 * VSTR + h * VSTR
                wv_view = bass.AP(tensor=vS.tensor, offset=off,
                                  ap=[list(vS.ap[0]), [1, 65]])
                nc.tensor.matmul(
                    cur["o"][par][0:65, :], wv_view, e[:, par],
                    start=(ki == 0), stop=(ki == KT - 1))
            return ki

        def attn_step(hp, qc, ki, g):
            assert ("k", hp, ki // 4) in qk_emitted, (hp, qc, ki, g)
            assert ("q", hp, qc) in qk_emitted, (hp, qc, ki, g)
            s_ps = spool.tile([P, 2, 512], F32, tag=f"s{g % 2}", name="s")
            for par in range(2):
                pb = par * 64
                nc.tensor.matmul(
                    s_ps[:, par],
                    kTc[hp][ki // 4][pb:pb + 64,
                                     (ki % 4) * P:(ki % 4 + 1) * P],
                    qTc[hp][qc][pb:pb + 64, :],
                    start=True, stop=True, tile_position=(pb, 0))
            e = epool.tile([P, 2, 512], BF16, tag="e", name="e")
            nc.scalar.activation(e, s_ps, Exp, scale=float(D) ** -0.5)
            pend.append((hp * NCH + qc, ki, e))

        def pops(g, cap=2):
            # hard: e-ring safety (AV(j) emitted before exp(j+EPOOL))
            while len(pend) > EPOOL - 2:
                pop_av()
            # soft: keep the backlog near the target lag
            done = 0
            while (pend and done < cap and len(pend) > _soft_lag(g)
                   and v_emitted[pend[0][1]]):
                ki = pop_av()
                done += 1
                if ki == KT - 1:
                    break  # boundary breather: let stage copies drain

        # ---------------- filler queue ----------------
        # each item: (gate_blk_or_None, step_thunk)
        fillers = []

        def push(steps, gate=None):
            for s in steps:
                fillers.append((gate, s))

        push(qk_unit_steps("k", wk_sb, kTc, 0, 1))
        push(qk_unit_steps("k", wk_sb, kTc, 0, 2))
        push(qk_unit_steps("k", wk_sb, kTc, 0, 3))
        push(qk_unit_steps("q", wq_sb, qTc, 0, 1))
        push(qk_unit_steps("q", wq_sb, qTc, 0, 2))
        for c in range(NCH):
            for nt4 in range(4):
                push(v_unit_steps(c, nt4))              # t0..t15
            if c == 2:
                push(qk_unit_steps("q", wq_sb, qTc, 0, 3))
        push(qk_unit_steps("k", wk_sb, kTc, 1, 0))
        push(qk_unit_steps("q", wq_sb, qTc, 1, 0))
        push(qk_unit_steps("k", wk_sb, kTc, 1, 1))
        push(qk_unit_steps("q", wq_sb, qTc, 1, 1))
        push(qk_unit_steps("k", wk_sb, kTc, 1, 2))
        push(qk_unit_steps("k", wk_sb, kTc, 1, 3))
        push(qk_unit_steps("q", wq_sb, qTc, 1, 2))
        push(qk_unit_steps("q", wq_sb, qTc, 1, 3))
        for t in (2, 3):
            for c in range(NCH):
                push(qk_unit_steps("k", wk_sb, kTc, t, c))
            for c in range(NCH):
                push(qk_unit_steps("q", wq_sb, qTc, t, c))
        # proj: partials pc_j gated on block (2,j); finals fc_j gated on
        # block (3,j).  Order [pc0 pc1 fc0 pc2 fc1 pc3 fc2] + fc3 at the
        # drain: every pp-slot-reusing write (pc2 reuses pc0 slots, pc3
        # reuses pc1 slots) is emitted after the final that reads them,
        # keeping the in-order DVE queue deadlock-free.
        def push_chunk_partials(j):
            for nt in range(j * 4, j * 4 + 4):
                for fc in range(2):
                    push(proj_partial_steps(nt, fc), gate=2 * NCH + j)

        def push_chunk_finals(j):
            for nt in range(j * 4, j * 4 + 4):
                for fc in range(2):
                    push(proj_final_step(nt, fc), gate=3 * NCH + j)

        push_chunk_partials(0)
        push_chunk_partials(1)
        push_chunk_finals(0)
        push_chunk_partials(2)
        push_chunk_finals(1)
        push_chunk_partials(3)
        push_chunk_finals(2)

        def emit_fillers(budget):
            n = 0
            while n < budget and fillers:
                gate, step = fillers[0]
                if gate is not None:
                    if gate not in finished:
                        break
                    # give the finish's DVE chain 2 ki of slack before the
                    # dependent proj matmuls hit the PE queue
                    if gnow[0] < finished_at.get(gate, 0) + 3:
                        break
                fillers.pop(0)
                step()
                n += 1
            return n

        # ---------------- head + main pacer ----------------
        for s in qk_unit_steps("k", wk_sb, kTc, 0, 0):
            s()
        for s in qk_unit_steps("q", wq_sb, qTc, 0, 0):
            s()

        for g in range(NBLK * KT):
            gnow[0] = g
            hp, qc, ki = g // 64, (g // 16) % 4, g % 16
            pops(g)
            attn_step(hp, qc, ki, g)
            emit_fillers(_budget(g))

        # drain: remaining AVs, gated projs, last finish, chunk-3 finals
        while pend:
            gnow[0] += 1
            pop_av()
            emit_fillers(2)
        gnow[0] += 1000
        # last block: latency-optimized finish (everything after it is
        # the serial dependency chain finish -> fc3 matmul -> add -> dma)
        o = cur["o"]
        rbs = []
        for par in range(2):
            dd = finp.tile([P, 512], F32, tag="dd", name="dd")
            nc.vector.tensor_copy(dd[0:1, :], o[par][64:65, :])
            r = finp.tile([P, 512], F32, tag="r", name="r")
            nc.vector.reciprocal_approx_fast(r[0:1, :], dd[0:1, :])
            rb = finp.tile([P, 512], F32, tag="rb", name="rb")
            nc.gpsimd.partition_broadcast(rb[0:64, :], r[0:1, :],
                                          channels=64)
            rbs.append(rb)
        for par in range(2):
            nc.vector.tensor_mul(
                aoTc[DHT - 1][NCH - 1][par * 64:par * 64 + 64, :],
                o[par][0:64, :], rbs[par][0:64, :])
        finished.add(NBLK - 1)
        emit_fillers(10 ** 9)
        assert not fillers, f"{len(fillers)} fillers never emitted"
        # chunk-3 finals: 8 matmuls into borrowed psum slots first (they
        # pipeline on the PE), then the adds + output DMAs
        slots = []
        for tag in ("s0", "s1"):
            sps = spool.tile([P, 2, 512], F32, tag=tag, name="s")
            slots += [sps[:, 0], sps[:, 1]]
        slots += [upool.tile([P, 512], F32, tag="ups", name="ups")
                  for _ in range(2)]
        slots += [opool.tile([P, 512], F32, tag=f"o{i}", name=f"o{i}")
                  for i in range(2)]
        ct = DHT - 1
        pairs = [(nt, fc) for nt in range((NCH - 1) * 4, NCH * 4)
                 for fc in range(2)]
        for (nt, fc), ps in zip(pairs, slots):
            nc.tensor.matmul(
                ps[0:128, :],
                aoTc[ct][nt // 4][:, (nt % 4) * P:(nt % 4 + 1) * P],
                wp_sb[ct][:, fc * 512:(fc + 1) * 512],
                start=True, stop=True)
        for (nt, fc), ps in zip(pairs, slots):
            ob = obp.tile([P, 512], BF16, tag="ob", name="ob")
            nc.vector.tensor_add(ob, pp_view(nt, fc), ps[0:128, :])
            nc.sync.dma_start(
                out[nt * P:(nt + 1) * P, fc * 512:(fc + 1) * 512], ob)


def build_nc():
    if "nc" in _CACHE:
        return _CACHE["nc"]
    import concourse.bass as bass
    import concourse.tile as tile
    from concourse import bacc, mybir

    nc = bacc.Bacc("TRN2", target_bir_lowering=False, debug=False,
                   enable_asserts=False, num_devices=NCORES)
    with tile.TileContext(nc) as tc:
        _emit(nc, tc, mybir, bass, tile)
    nc.compile()
    _CACHE["nc"] = nc
    return nc


def _perm128():
    """new_row -> old_row permutation within a 128-channel head pair."""
    perm = np.zeros(P, dtype=np.int64)
    sign = np.zeros(P, dtype=np.float32)
    dmap = np.zeros(P, dtype=np.int64)
    for new in range(P):
        Q, p = new // 32, new % 32
        h = Q // 2
        base = (Q % 2) * 16
        d = base + (32 if p >= 16 else 0) + (p % 16)
        perm[new] = h * 64 + d
        dmap[new] = d
        sign[new] = -1.0 if p < 16 else 1.0
    return perm, dmap, sign


def make_in_maps(x, rope_cos, rope_sin, w_qkv, w_proj):
    import ml_dtypes
    BF = ml_dtypes.bfloat16
    x = np.asarray(x, dtype=np.float32)
    rope_cos = np.asarray(rope_cos, dtype=np.float32)
    rope_sin = np.asarray(rope_sin, dtype=np.float32)
    w_qkv = np.asarray(w_qkv, dtype=np.float32)
    w_proj = np.asarray(w_proj, dtype=np.float32)

    perm, dmap, sign = _perm128()
    colperm = np.concatenate([t * P + perm for t in range(DHT)])

    cosp = np.ascontiguousarray(rope_cos.T[dmap, :]).astype(BF)   # [128, N]
    ssin = np.ascontiguousarray(
        rope_sin.T[dmap, :] * sign[:, None]).astype(BF)

    xTs = [np.ascontiguousarray(x[b].T).astype(BF) for b in range(B)]

    in_maps = []
    for core in range(NCORES):
        b = core // 2
        hg = core % 2
        wq_c = np.ascontiguousarray(
            w_qkv[hg * DH:(hg + 1) * DH, :].T[:, colperm]).astype(BF)
        wk_c = np.ascontiguousarray(
            w_qkv[C + hg * DH:C + (hg + 1) * DH, :].T[:, colperm]).astype(BF)
        in_maps.append({
            "xT": xTs[b],
            "wq": wq_c,
            "wk": wk_c,
            "wv": np.ascontiguousarray(
                w_qkv[2 * C + hg * DH:2 * C + (hg + 1) * DH, :].T).astype(BF),
            "wp": np.ascontiguousarray(
                w_proj[:, hg * DH:(hg + 1) * DH].T).astype(BF),
            "cosp": cosp,
            "ssin": ssin,
            "onesd": np.ones((P, P), dtype=BF),
        })
    return in_maps


def kernel(x, rope_cos, rope_sin, w_qkv, w_proj, b_proj, trace=False):
    from concourse.bass_utils import run_bass_kernel_spmd

    nc = build_nc()
    in_maps = make_in_maps(x, rope_cos, rope_sin, w_qkv, w_proj)
    res = run_bass_kernel_spmd(nc, in_maps, core_ids=list(range(NCORES)),
                               trace=trace)
    b_proj = np.asarray(b_proj, dtype=np.float32)
    final = np.empty((B, N, C), dtype=np.float32)
    for b in range(B):
        final[b] = (res.results[2 * b]["out"].astype(np.float32)
                    + res.results[2 * b + 1]["out"].astype(np.float32)
                    + b_proj)
    if trace:
        kernel.last_exec_time_ns = res.exec_time_ns
        kernel.last_results = res
    return final


# revision 30
# speedup vs baseline: 1.0038x; 1.0038x over previous
"""Fused multi-head attention (qkv + RoPE + softmax + proj) for TRN2, 8 cores.

Sharding: core c -> batch b=c//2, head group hg=c%2 (8 of 16 heads).
Data-parallel over B (4), 2-way tensor-parallel over heads.
Host unshard: out[b] = partial[2b] + partial[2b+1] + b_proj.

v5 (411us -> ~381us): PE-first schedule.  Profiling showed the tensor
engine is the bottleneck (341us busy), not the exp stream (277us):
  1. RoPE rotate-half runs as a DVE stream_shuffle instead of a PE
     matmul: head-dim channels are permuted host-side so each rope pair
     (d, d+32) lands in one 32-partition quadrant; the sign is folded
     into the sin table.  Kills 32 PE matmuls + 32 PSUM casts + the p2t
     constant + one PSUM bank.
  2. Every producer unit (q/k/v/proj) is split into single-matmul steps
     fed by a per-ki pacer whose budget LEVELS the PE load at ~4.5
     units/ki: heavy early (the v/k/q front-load the first block needs),
     1/ki through the ACT-bound middle, 2/ki in the proj-heavy back.
  3. AV matmuls drain from an elastic FIFO (e-ring EPOOL=20).
     Emission-order rule (engine queues are in-order, a cycle
     deadlocks): AV(j) must be emitted before exp(j+EPOOL) reuses its
     e-slot, which sets every v-unit's emission deadline; asserts
     enforce them at build time.
  4. proj is split: hp0-2 partial sums run during the ACT-bound middle
     (gated on the hp2 block of their chunk) and park in f32 bitcast
     views of the dead xs memory; the hp3 final step adds partial+psum.
     Slot reuse order [pc0 pc1 fc0 pc2 fc1 pc3 fc2] keeps the in-order
     DVE queue deadlock-free.
  5. Batched DMA (~30 issues vs 133; the sync engine takes ~600ns per
     issue), ordered so the first k-unit starts ~9us in; x chunk 0 is
     split into 4 pieces so matmuls chase the DMA.
  6. Tail: lag taper, latency-optimized last-block normalize, and the 8
     final proj matmuls go to borrowed psum slots before their adds.
Steady state: ACT-bound middle at ~1078ns/ki, PE-bound front/back.
bf16 everywhere (incl cos/sin tables; numpy-sim rel err 1.1e-2 vs the
2e-2 gate); rope arithmetic f32 in PSUM.
"""

import sys

if "/opt/trn_rl_repo" not in sys.path:
    sys.path.insert(0, "/opt/trn_rl_repo")

import numpy as np
from contextlib import ExitStack

B, N, C, H, D = 4, 2048, 1024, 16, 64
NCORES = 8
P = 128
DH = 512          # per-core head channels (8 heads x 64)
CT = C // P       # 8 contraction tiles for qkv
DHT = DH // P     # 4 partition tiles of qT/kT/aoT (= head pairs)
NCH = N // 512    # 4 n chunks of 512
KT = N // P       # 16 key tiles
NBLK = DHT * NCH  # 16 attention blocks
EPOOL = 20        # e-tile ring; AV lag hard bound = EPOOL - 2
VSTR = 72         # per-head stride in the packed v tile (64 d + ones + pad)

# stream_shuffle mask: within each 32-partition quadrant swap the 16-halves
SHUF_MASK = list(range(16, 32)) + list(range(16))

_CACHE = {}


def _soft_lag(g):
    if g >= 246:
        return 1
    if g >= 240:
        return 2
    if g >= 224:
        return 3
    if g >= 192:
        return 6
    return 15


def _budget(g):
    if g < 24:
        return 6
    if g < 36:
        return 4
    if g < 62:
        return 2
    if g < 160:
        return 1
    return 2


def _emit(nc, tc, mybir, bass, tile):
    F32 = mybir.dt.float32
    BF16 = mybir.dt.bfloat16
    Exp = mybir.ActivationFunctionType.Exp

    xT = nc.dram_tensor("xT", [C, N], BF16, kind="ExternalInput").ap()
    wq = nc.dram_tensor("wq", [C, DH], BF16, kind="ExternalInput").ap()
    wk = nc.dram_tensor("wk", [C, DH], BF16, kind="ExternalInput").ap()
    wv = nc.dram_tensor("wv", [C, DH], BF16, kind="ExternalInput").ap()
    wp = nc.dram_tensor("wp", [DH, C], BF16, kind="ExternalInput").ap()
    cosp = nc.dram_tensor("cosp", [P, N], BF16, kind="ExternalInput").ap()
    ssin = nc.dram_tensor("ssin", [P, N], BF16, kind="ExternalInput").ap()
    onesd = nc.dram_tensor("onesd", [P, P], BF16, kind="ExternalInput").ap()
    out = nc.dram_tensor("out", [N, C], BF16, kind="ExternalOutput").ap()

    ctx = ExitStack()
    with ctx:
        consts = ctx.enter_context(tc.tile_pool(name="consts", bufs=1))
        persist = ctx.enter_context(tc.tile_pool(name="persist", bufs=1))

        cos_c = [consts.tile([P, 512], BF16, tag=f"cos{c}", name=f"cos{c}")
                 for c in range(NCH)]
        sin_c = [consts.tile([P, 512], BF16, tag=f"sin{c}", name=f"sin{c}")
                 for c in range(NCH)]

        qTc = [[persist.tile([P, 512], BF16, tag=f"qT{t}_{c}",
                             name=f"qT{t}_{c}")
                for c in range(NCH)] for t in range(DHT)]
        kTc = [[persist.tile([P, 512], BF16, tag=f"kT{t}_{c}",
                             name=f"kT{t}_{c}")
                for c in range(NCH)] for t in range(DHT)]
        aoTc = [[persist.tile([P, 512], BF16, tag=f"aoT{t}_{c}",
                              name=f"aoT{t}_{c}")
                 for c in range(NCH)] for t in range(DHT)]
        # one packed tile for all v key tiles: [128 keys, 16 ki x 8 h x 72]
        vS = persist.tile([P, KT * 8 * VSTR], BF16, tag="vS", name="vS")
        wp_sb = [persist.tile([P, C], BF16, tag=f"wp{i}", name=f"wp{i}")
                 for i in range(DHT)]
        # per head-pair qkv weights: [128 in-ch, 8 kc x 128 out-ch]
        wq_sb = [persist.tile([P, CT * P], BF16, tag=f"wq{t}", name=f"wq{t}")
                 for t in range(DHT)]
        wk_sb = [persist.tile([P, CT * P], BF16, tag=f"wk{t}", name=f"wk{t}")
                 for t in range(DHT)]
        wv_sb = persist.tile([P, CT * DH], BF16, tag="wv", name="wv")
        # x chunks: [128 in-ch, 8 kc x 512 n]
        xs = [persist.tile([P, CT * 512], BF16, tag=f"x{c}", name=f"x{c}")
              for c in range(NCH)]

        upool = ctx.enter_context(tc.tile_pool(name="upool", bufs=2,
                                               space="PSUM"))
        spool = ctx.enter_context(tc.tile_pool(name="spool", bufs=1,
                                               space="PSUM"))
        opool = ctx.enter_context(tc.tile_pool(name="opool", bufs=1,
                                               space="PSUM"))
        epool = ctx.enter_context(tc.tile_pool(name="epool", bufs=EPOOL))
        shufp = ctx.enter_context(tc.tile_pool(name="shufp", bufs=1))
        ttp = ctx.enter_context(tc.tile_pool(name="ttp", bufs=1))
        finp = ctx.enter_context(tc.tile_pool(name="finp", bufs=2))
        obp = ctx.enter_context(tc.tile_pool(name="obp", bufs=2))

        # ---------------- DMA (batched, first-need-first) ----------------
        def dma_x(c, kc0, kc1):
            dst = bass.AP(tensor=xs[c].tensor, offset=kc0 * 512,
                          ap=[list(xs[c].ap[0]), [512, kc1 - kc0], [1, 512]])
            src = bass.AP(tensor=xT.tensor, offset=kc0 * P * N + c * 512,
                          ap=[[N, P], [P * N, kc1 - kc0], [1, 512]])
            nc.sync.dma_start(dst, src)

        def dma_w(dst_tile, w_ap, t):
            dst = bass.AP(tensor=dst_tile.tensor, offset=0,
                          ap=[list(dst_tile.ap[0]), [P, CT], [1, P]])
            src = bass.AP(tensor=w_ap.tensor, offset=t * P,
                          ap=[[DH, P], [P * DH, CT], [1, P]])
            nc.sync.dma_start(dst, src)

        # first k-unit's tiles land first, in small pieces so matmuls can
        # start while the rest streams; x chunks 1-3 (big, ring-clogging)
        # go after everything the front needs
        dma_x(0, 0, 1)
        dma_w(wk_sb[0], wk, 0)
        dma_x(0, 1, 2)
        dma_x(0, 2, 4)
        dma_w(wq_sb[0], wq, 0)
        dma_x(0, 4, 8)
        dma_x(1, 0, 4)
        dma_x(1, 4, 8)
        nc.sync.dma_start(cos_c[0], cosp[:, 0:512])
        nc.sync.dma_start(sin_c[0], ssin[:, 0:512])
        wv_dst = bass.AP(tensor=wv_sb.tensor, offset=0,
                         ap=[list(wv_sb.ap[0]), [DH, CT], [1, DH]])
        wv_src = bass.AP(tensor=wv.tensor, offset=0,
                         ap=[[DH, P], [P * DH, CT], [1, DH]])
        nc.sync.dma_start(wv_dst, wv_src)
        dma_x(2, 0, 4)
        dma_x(2, 4, 8)
        nc.sync.dma_start(cos_c[1], cosp[:, 512:1024])
        nc.sync.dma_start(sin_c[1], ssin[:, 512:1024])
        dma_x(3, 0, 4)
        dma_x(3, 4, 8)
        for c in range(2, NCH):
            nc.sync.dma_start(cos_c[c], cosp[:, c * 512:(c + 1) * 512])
            nc.sync.dma_start(sin_c[c], ssin[:, c * 512:(c + 1) * 512])
        # ones columns into vS: offset 64 within each 72-stride head block
        ones_dst = bass.AP(tensor=vS.tensor, offset=64,
                           ap=[list(vS.ap[0]), [8 * VSTR, KT], [VSTR, 8]])
        ones_src = bass.AP(tensor=onesd.tensor, offset=0,
                           ap=[[P, P], [8, KT], [1, 8]])
        nc.sync.dma_start(ones_dst, ones_src)
        for t in range(1, DHT):
            dma_w(wk_sb[t], wk, t)
            dma_w(wq_sb[t], wq, t)
        for i in range(DHT):
            nc.sync.dma_start(wp_sb[i], wp[i * P:(i + 1) * P, :])

        # ---------------- emission-order tracking ----------------
        v_emitted = [False] * KT
        qk_emitted = set()   # ("q"|"k", t, c)

        # ---------------- unit step factories ----------------
        def qk_unit_steps(kind, w_sb, dst, t, c):
            hold = {}

            def step(kc):
                def f():
                    if kc == 0:
                        hold["ps"] = upool.tile([P, 512], F32, tag="ups", name="ups")
                    ps = hold["ps"]
                    nc.tensor.matmul(
                        ps, w_sb[t][:, kc * P:(kc + 1) * P],
                        xs[c][:, kc * 512:(kc + 1) * 512],
                        start=(kc == 0), stop=(kc == CT - 1))
                    if kc == CT - 1:
                        shuf = shufp.tile([P, 512], F32, tag="shuf", name="shuf")
                        nc.vector.stream_shuffle(shuf, ps, SHUF_MASK)
                        t1 = ttp.tile([P, 512], F32, tag="t1", name="t1")
                        nc.vector.tensor_mul(t1, ps, cos_c[c])
                        t2 = ttp.tile([P, 512], F32, tag="t2", name="t2")
                        nc.vector.tensor_mul(t2, shuf, sin_c[c])
                        nc.vector.tensor_add(dst[t][c], t1, t2)
                        qk_emitted.add((kind, t, c))
                return f
            return [step(kc) for kc in range(CT)]

        def v_unit_steps(c, nt4):
            i = c * 4 + nt4
            hold = {}

            def step(kc):
                def f():
                    if kc == 0:
                        hold["ps"] = upool.tile([P, 512], F32, tag="ups", name="ups")
                    ps = hold["ps"]
                    nc.tensor.matmul(
                        ps,
                        xs[c][:, kc * 512 + nt4 * P:kc * 512 + (nt4 + 1) * P],
                        wv_sb[:, kc * DH:(kc + 1) * DH],
                        start=(kc == 0), stop=(kc == CT - 1))
                    if kc == CT - 1:
                        v_view = bass.AP(
                            tensor=vS.tensor, offset=i * 8 * VSTR,
                            ap=[list(vS.ap[0]), [VSTR, 8], [1, 64]])
                        nc.vector.tensor_copy(
                            v_view, ps.rearrange("p (h d) -> p h d", h=8))
                        v_emitted[i] = True
                return f
            return [step(kc) for kc in range(CT)]

        # proj is split: hp0-2 partial sums run early (gated on the hp2
        # block of their chunk) and park in f32 views of the dead xs
        # memory; the hp3 final step adds partial + psum -> out.
        def pp_view(nt, fc):
            j = nt // 4
            slot = (j % 2) * 8 + (nt % 4) * 2 + fc
            base = xs[slot // 4][:, 0:CT * 512].bitcast(F32)
            return base[:, (slot % 4) * 512:(slot % 4 + 1) * 512]

        def proj_partial_steps(nt, fc):
            hold = {}

            def step(ct):
                def f():
                    if ct == 0:
                        hold["ps"] = upool.tile([P, 512], F32, tag="ups", name="ups")
                    ps = hold["ps"]
                    nc.tensor.matmul(
                        ps,
                        aoTc[ct][nt // 4][:, (nt % 4) * P:(nt % 4 + 1) * P],
                        wp_sb[ct][:, fc * 512:(fc + 1) * 512],
                        start=(ct == 0), stop=(ct == DHT - 2))
                    if ct == DHT - 2:
                        nc.vector.tensor_copy(pp_view(nt, fc), ps)
                return f
            return [step(ct) for ct in range(DHT - 1)]

        def proj_final_step(nt, fc):
            def f():
                ct = DHT - 1
                ps = upool.tile([P, 512], F32, tag="ups", name="ups")
                nc.tensor.matmul(
                    ps,
                    aoTc[ct][nt // 4][:, (nt % 4) * P:(nt % 4 + 1) * P],
                    wp_sb[ct][:, fc * 512:(fc + 1) * 512],
                    start=True, stop=True)
                ob = obp.tile([P, 512], BF16, tag="ob", name="ob")
                nc.vector.tensor_add(ob, pp_view(nt, fc), ps)
                nc.sync.dma_start(
                    out[nt * P:(nt + 1) * P, fc * 512:(fc + 1) * 512], ob)
            return [f]

        # ---------------- attention primitives ----------------
        pend = []          # FIFO of (blk, ki, e_tile)
        cur = {"blk": None, "o": None}
        finished = set()
        finished_at = {}
        gnow = [0]

        def finish_block(blk, o):
            hp, qc = blk // NCH, blk % NCH
            for par in range(2):
                pb = par * 64
                stage = finp.tile([P, 512], F32, tag="stg", name="stg")
                nc.vector.tensor_copy(stage[0:64, :], o[par][0:64, :])
                dd = finp.tile([P, 512], F32, tag="dd", name="dd")
                nc.vector.tensor_copy(dd[0:1, :], o[par][64:65, :])
                r = finp.tile([P, 512], F32, tag="r", name="r")
                nc.vector.reciprocal_approx_fast(r[0:1, :], dd[0:1, :])
                rb = finp.tile([P, 512], F32, tag="rb", name="rb")
                nc.gpsimd.partition_broadcast(
                    rb[0:64, :], r[0:1, :], channels=64)
                nc.vector.tensor_mul(
                    aoTc[hp][qc][pb:pb + 64, :], stage[0:64, :],
                    rb[0:64, :])
            finished.add(blk)
            finished_at[blk] = gnow[0]

        def pop_av():
            blk, ki, e = pend.pop(0)
            assert v_emitted[ki], f"v[{ki}] not emitted before AV pop"
            if blk != cur["blk"]:
                if cur["blk"] is not None:
                    finish_block(cur["blk"], cur["o"])
                cur["blk"] = blk
                cur["o"] = {par: opool.tile([P, 512], F32, tag=f"o{par}",
                                            name=f"o{par}")
                            for par in range(2)}
            for par in range(2):
                h = (blk // NCH) * 2 + par
                off = ki * 8 * VSTR + h * VSTR
                wv_view = bass.AP(tensor=vS.tensor, offset=off,
                                  ap=[list(vS.ap[0]), [1, 65]])
                nc.tensor.matmul(
                    cur["o"][par][0:65, :], wv_view, e[:, par],
                    start=(ki == 0), stop=(ki == KT - 1))
            return ki

        def attn_step(hp, qc, ki, g):
            assert ("k", hp, ki // 4) in qk_emitted, (hp, qc, ki, g)
            assert ("q", hp, qc) in qk_emitted, (hp, qc, ki, g)
            s_ps = spool.tile([P, 2, 512], F32, tag=f"s{g % 2}", name="s")
            for par in range(2):
                pb = par * 64
                nc.tensor.matmul(
                    s_ps[:, par],
                    kTc[hp][ki // 4][pb:pb + 64,
                                     (ki % 4) * P:(ki % 4 + 1) * P],
                    qTc[hp][qc][pb:pb + 64, :],
                    start=True, stop=True, tile_position=(pb, 0))
            e = epool.tile([P, 2, 512], BF16, tag="e", name="e")
            nc.scalar.activation(e, s_ps, Exp, scale=float(D) ** -0.5)
            pend.append((hp * NCH + qc, ki, e))

        def pops(g, cap=2):
            # hard: e-ring safety (AV(j) emitted before exp(j+EPOOL))
            while len(pend) > EPOOL - 2:
                pop_av()
            # soft: keep the backlog near the target lag
            done = 0
            while (pend and done < cap and len(pend) > _soft_lag(g)
                   and v_emitted[pend[0][1]]):
                ki = pop_av()
                done += 1
                if ki == KT - 1:
                    break  # boundary breather: let stage copies drain

        # ---------------- filler queue ----------------
        # each item: (gate_blk_or_None, step_thunk)
        fillers = []

        def push(steps, gate=None):
            for s in steps:
                fillers.append((gate, s))

        push(qk_unit_steps("k", wk_sb, kTc, 0, 1))
        push(qk_unit_steps("k", wk_sb, kTc, 0, 2))
        push(qk_unit_steps("k", wk_sb, kTc, 0, 3))
        push(qk_unit_steps("q", wq_sb, qTc, 0, 1))
        push(qk_unit_steps("q", wq_sb, qTc, 0, 2))
        for c in range(NCH):
            for nt4 in range(4):
                push(v_unit_steps(c, nt4))              # t0..t15
            if c == 2:
                push(qk_unit_steps("q", wq_sb, qTc, 0, 3))
        push(qk_unit_steps("k", wk_sb, kTc, 1, 0))
        push(qk_unit_steps("q", wq_sb, qTc, 1, 0))
        push(qk_unit_steps("k", wk_sb, kTc, 1, 1))
        push(qk_unit_steps("q", wq_sb, qTc, 1, 1))
        push(qk_unit_steps("k", wk_sb, kTc, 1, 2))
        push(qk_unit_steps("k", wk_sb, kTc, 1, 3))
        push(qk_unit_steps("q", wq_sb, qTc, 1, 2))
        push(qk_unit_steps("q", wq_sb, qTc, 1, 3))
        for t in (2, 3):
            for c in range(NCH):
                push(qk_unit_steps("k", wk_sb, kTc, t, c))
            for c in range(NCH):
                push(qk_unit_steps("q", wq_sb, qTc, t, c))
        # proj: partials pc_j gated on block (2,j); finals fc_j gated on
        # block (3,j).  Order [pc0 pc1 fc0 pc2 fc1 pc3 fc2] + fc3 at the
        # drain: every pp-slot-reusing write (pc2 reuses pc0 slots, pc3
        # reuses pc1 slots) is emitted after the final that reads them,
        # keeping the in-order DVE queue deadlock-free.
        def push_chunk_partials(j):
            for nt in range(j * 4, j * 4 + 4):
                for fc in range(2):
                    push(proj_partial_steps(nt, fc), gate=2 * NCH + j)

        def push_chunk_finals(j):
            for nt in range(j * 4, j * 4 + 4):
                for fc in range(2):
                    push(proj_final_step(nt, fc), gate=3 * NCH + j)

        push_chunk_partials(0)
        push_chunk_partials(1)
        push_chunk_finals(0)
        push_chunk_partials(2)
        push_chunk_finals(1)
        push_chunk_partials(3)
        push_chunk_finals(2)

        def emit_fillers(budget):
            n = 0
            while n < budget and fillers:
                gate, step = fillers[0]
                if gate is not None:
                    if gate not in finished:
                        break
                    # give the finish's DVE chain 2 ki of slack before the
                    # dependent proj matmuls hit the PE queue
                    if gnow[0] < finished_at.get(gate, 0) + 2:
                        break
                fillers.pop(0)
                step()
                n += 1
            return n

        # ---------------- head + main pacer ----------------
        for s in qk_unit_steps("k", wk_sb, kTc, 0, 0):
            s()
        for s in qk_unit_steps("q", wq_sb, qTc, 0, 0):
            s()

        for g in range(NBLK * KT):
            gnow[0] = g
            hp, qc, ki = g // 64, (g // 16) % 4, g % 16
            pops(g)
            attn_step(hp, qc, ki, g)
            emit_fillers(_budget(g))

        # drain: remaining AVs, gated projs, last finish, chunk-3 finals
        while pend:
            gnow[0] += 1
            pop_av()
            emit_fillers(2)
        gnow[0] += 1000
        # last block: latency-optimized finish (everything after it is
        # the serial dependency chain finish -> fc3 matmul -> add -> dma)
        o = cur["o"]
        rbs = []
        for par in range(2):
            dd = finp.tile([P, 512], F32, tag="dd", name="dd")
            nc.vector.tensor_copy(dd[0:1, :], o[par][64:65, :])
            r = finp.tile([P, 512], F32, tag="r", name="r")
            nc.vector.reciprocal_approx_fast(r[0:1, :], dd[0:1, :])
            rb = finp.tile([P, 512], F32, tag="rb", name="rb")
            nc.gpsimd.partition_broadcast(rb[0:64, :], r[0:1, :],
                                          channels=64)
            rbs.append(rb)
        for par in range(2):
            nc.vector.tensor_mul(
                aoTc[DHT - 1][NCH - 1][par * 64:par * 64 + 64, :],
                o[par][0:64, :], rbs[par][0:64, :])
        finished.add(NBLK - 1)
        emit_fillers(10 ** 9)
        assert not fillers, f"{len(fillers)} fillers never emitted"
        # chunk-3 finals: 8 matmuls into borrowed psum slots first (they
        # pipeline on the PE), then the adds + output DMAs
        slots = []
        for tag in ("s0", "s1"):
            sps = spool.tile([P, 2, 512], F32, tag=tag, name="s")
            slots += [sps[:, 0], sps[:, 1]]
        slots += [upool.tile([P, 512], F32, tag="ups", name="ups")
                  for _ in range(2)]
        slots += [opool.tile([P, 512], F32, tag=f"o{i}", name=f"o{i}")
                  for i in range(2)]
        ct = DHT - 1
        pairs = [(nt, fc) for nt in range((NCH - 1) * 4, NCH * 4)
                 for fc in range(2)]
        for (nt, fc), ps in zip(pairs, slots):
            nc.tensor.matmul(
                ps[0:128, :],
                aoTc[ct][nt // 4][:, (nt % 4) * P:(nt % 4 + 1) * P],
                wp_sb[ct][:, fc * 512:(fc + 1) * 512],
                start=True, stop=True)
        for (nt, fc), ps in zip(pairs, slots):
            ob = obp.tile([P, 512], BF16, tag="ob", name="ob")
            nc.vector.tensor_add(ob, pp_view(nt, fc), ps[0:128, :])
            nc.sync.dma_start(
                out[nt * P:(nt + 1) * P, fc * 512:(fc + 1) * 512], ob)


def build_nc():
    if "nc" in _CACHE:
        return _CACHE["nc"]
    import concourse.bass as bass
    import concourse.tile as tile
    from concourse import bacc, mybir

    nc = bacc.Bacc("TRN2", target_bir_lowering=False, debug=False,
                   enable_asserts=False, num_devices=NCORES)
    with tile.TileContext(nc) as tc:
        _emit(nc, tc, mybir, bass, tile)
    nc.compile()
    _CACHE["nc"] = nc
    return nc


def _perm128():
    """new_row -> old_row permutation within a 128-channel head pair."""
    perm = np.zeros(P, dtype=np.int64)
    sign = np.zeros(P, dtype=np.float32)
    dmap = np.zeros(P, dtype=np.int64)
    for new in range(P):
        Q, p = new // 32, new % 32
        h = Q // 2
        base = (Q % 2) * 16
        d = base + (32 if p >= 16 else 0) + (p % 16)
        perm[new] = h * 64 + d
        dmap[new] = d
        sign[new] = -1.0 if p < 16 else 1.0
    return perm, dmap, sign


def make_in_maps(x, rope_cos, rope_sin, w_qkv, w_proj):
    import ml_dtypes
    BF = ml_dtypes.bfloat16
    x = np.asarray(x, dtype=np.float32)
    rope_cos = np.asarray(rope_cos, dtype=np.float32)
    rope_sin = np.asarray(rope_sin, dtype=np.float32)
    w_qkv = np.asarray(w_qkv, dtype=np.float32)
    w_proj = np.asarray(w_proj, dtype=np.float32)

    perm, dmap, sign = _perm128()
    colperm = np.concatenate([t * P + perm for t in range(DHT)])

    cosp = np.ascontiguousarray(rope_cos.T[dmap, :]).astype(BF)   # [128, N]
    ssin = np.ascontiguousarray(
        rope_sin.T[dmap, :] * sign[:, None]).astype(BF)

    xTs = [np.ascontiguousarray(x[b].T).astype(BF) for b in range(B)]

    in_maps = []
    for core in range(NCORES):
        b = core // 2
        hg = core % 2
        wq_c = np.ascontiguousarray(
            w_qkv[hg * DH:(hg + 1) * DH, :].T[:, colperm]).astype(BF)
        wk_c = np.ascontiguousarray(
            w_qkv[C + hg * DH:C + (hg + 1) * DH, :].T[:, colperm]).astype(BF)
        in_maps.append({
            "xT": xTs[b],
            "wq": wq_c,
            "wk": wk_c,
            "wv": np.ascontiguousarray(
                w_qkv[2 * C + hg * DH:2 * C + (hg + 1) * DH, :].T).astype(BF),
            "wp": np.ascontiguousarray(
                w_proj[:, hg * DH:(hg + 1) * DH].T).astype(BF),
            "cosp": cosp,
            "ssin": ssin,
            "onesd": np.ones((P, P), dtype=BF),
        })
    return in_maps


def kernel(x, rope_cos, rope_sin, w_qkv, w_proj, b_proj, trace=False):
    from concourse.bass_utils import run_bass_kernel_spmd

    nc = build_nc()
    in_maps = make_in_maps(x, rope_cos, rope_sin, w_qkv, w_proj)
    res = run_bass_kernel_spmd(nc, in_maps, core_ids=list(range(NCORES)),
                               trace=trace)
    b_proj = np.asarray(b_proj, dtype=np.float32)
    final = np.empty((B, N, C), dtype=np.float32)
    for b in range(B):
        final[b] = (res.results[2 * b]["out"].astype(np.float32)
                    + res.results[2 * b + 1]["out"].astype(np.float32)
                    + b_proj)
    if trace:
        kernel.last_exec_time_ns = res.exec_time_ns
        kernel.last_results = res
    return final


# revision 31
# speedup vs baseline: 1.0082x; 1.0044x over previous
"""Fused multi-head attention (qkv + RoPE + softmax + proj) for TRN2, 8 cores.

Sharding: core c -> batch b=c//2, head group hg=c%2 (8 of 16 heads).
Data-parallel over B (4), 2-way tensor-parallel over heads.
Host unshard: out[b] = partial[2b] + partial[2b+1] + b_proj.

v5 (411us -> ~381us): PE-first schedule.  Profiling showed the tensor
engine is the bottleneck (341us busy), not the exp stream (277us):
  1. RoPE rotate-half runs as a DVE stream_shuffle instead of a PE
     matmul: head-dim channels are permuted host-side so each rope pair
     (d, d+32) lands in one 32-partition quadrant; the sign is folded
     into the sin table.  Kills 32 PE matmuls + 32 PSUM casts + the p2t
     constant + one PSUM bank.
  2. Every producer unit (q/k/v/proj) is split into single-matmul steps
     fed by a per-ki pacer whose budget LEVELS the PE load at ~4.5
     units/ki: heavy early (the v/k/q front-load the first block needs),
     1/ki through the ACT-bound middle, 2/ki in the proj-heavy back.
  3. AV matmuls drain from an elastic FIFO (e-ring EPOOL=20).
     Emission-order rule (engine queues are in-order, a cycle
     deadlocks): AV(j) must be emitted before exp(j+EPOOL) reuses its
     e-slot, which sets every v-unit's emission deadline; asserts
     enforce them at build time.
  4. proj is split: hp0-2 partial sums run during the ACT-bound middle
     (gated on the hp2 block of their chunk) and park in f32 bitcast
     views of the dead xs memory; the hp3 final step adds partial+psum.
     Slot reuse order [pc0 pc1 fc0 pc2 fc1 pc3 fc2] keeps the in-order
     DVE queue deadlock-free.
  5. Batched DMA (~30 issues vs 133; the sync engine takes ~600ns per
     issue), ordered so the first k-unit starts ~9us in; x chunk 0 is
     split into 4 pieces so matmuls chase the DMA.
  6. Tail: lag taper, latency-optimized last-block normalize, and the 8
     final proj matmuls go to borrowed psum slots before their adds.
Steady state: ACT-bound middle at ~1078ns/ki, PE-bound front/back.
bf16 everywhere (incl cos/sin tables; numpy-sim rel err 1.1e-2 vs the
2e-2 gate); rope arithmetic f32 in PSUM.
"""

import sys

if "/opt/trn_rl_repo" not in sys.path:
    sys.path.insert(0, "/opt/trn_rl_repo")

import numpy as np
from contextlib import ExitStack

B, N, C, H, D = 4, 2048, 1024, 16, 64
NCORES = 8
P = 128
DH = 512          # per-core head channels (8 heads x 64)
CT = C // P       # 8 contraction tiles for qkv
DHT = DH // P     # 4 partition tiles of qT/kT/aoT (= head pairs)
NCH = N // 512    # 4 n chunks of 512
KT = N // P       # 16 key tiles
NBLK = DHT * NCH  # 16 attention blocks
EPOOL = 20        # e-tile ring; AV lag hard bound = EPOOL - 2
VSTR = 72         # per-head stride in the packed v tile (64 d + ones + pad)

# stream_shuffle mask: within each 32-partition quadrant swap the 16-halves
SHUF_MASK = list(range(16, 32)) + list(range(16))

_CACHE = {}


def _soft_lag(g):
    if g >= 246:
        return 1
    if g >= 240:
        return 2
    if g >= 224:
        return 3
    if g >= 192:
        return 6
    return 15


def _budget(g):
    if g < 24:
        return 6
    if g < 36:
        return 4
    if g < 62:
        return 2
    if g < 160:
        return 1
    return 2


def _emit(nc, tc, mybir, bass, tile):
    F32 = mybir.dt.float32
    BF16 = mybir.dt.bfloat16
    Exp = mybir.ActivationFunctionType.Exp

    xT = nc.dram_tensor("xT", [C, N], BF16, kind="ExternalInput").ap()
    wq = nc.dram_tensor("wq", [C, DH], BF16, kind="ExternalInput").ap()
    wk = nc.dram_tensor("wk", [C, DH], BF16, kind="ExternalInput").ap()
    wv = nc.dram_tensor("wv", [C, DH], BF16, kind="ExternalInput").ap()
    wp = nc.dram_tensor("wp", [DH, C], BF16, kind="ExternalInput").ap()
    cosp = nc.dram_tensor("cosp", [P, N], BF16, kind="ExternalInput").ap()
    ssin = nc.dram_tensor("ssin", [P, N], BF16, kind="ExternalInput").ap()
    onesd = nc.dram_tensor("onesd", [P, P], BF16, kind="ExternalInput").ap()
    out = nc.dram_tensor("out", [N, C], BF16, kind="ExternalOutput").ap()

    ctx = ExitStack()
    with ctx:
        consts = ctx.enter_context(tc.tile_pool(name="consts", bufs=1))
        persist = ctx.enter_context(tc.tile_pool(name="persist", bufs=1))

        cos_c = [consts.tile([P, 512], BF16, tag=f"cos{c}", name=f"cos{c}")
                 for c in range(NCH)]
        sin_c = [consts.tile([P, 512], BF16, tag=f"sin{c}", name=f"sin{c}")
                 for c in range(NCH)]

        qTc = [[persist.tile([P, 512], BF16, tag=f"qT{t}_{c}",
                             name=f"qT{t}_{c}")
                for c in range(NCH)] for t in range(DHT)]
        kTc = [[persist.tile([P, 512], BF16, tag=f"kT{t}_{c}",
                             name=f"kT{t}_{c}")
                for c in range(NCH)] for t in range(DHT)]
        aoTc = [[persist.tile([P, 512], BF16, tag=f"aoT{t}_{c}",
                              name=f"aoT{t}_{c}")
                 for c in range(NCH)] for t in range(DHT)]
        # one packed tile for all v key tiles: [128 keys, 16 ki x 8 h x 72]
        vS = persist.tile([P, KT * 8 * VSTR], BF16, tag="vS", name="vS")
        wp_sb = [persist.tile([P, C], BF16, tag=f"wp{i}", name=f"wp{i}")
                 for i in range(DHT)]
        # per head-pair qkv weights: [128 in-ch, 8 kc x 128 out-ch]
        wq_sb = [persist.tile([P, CT * P], BF16, tag=f"wq{t}", name=f"wq{t}")
                 for t in range(DHT)]
        wk_sb = [persist.tile([P, CT * P], BF16, tag=f"wk{t}", name=f"wk{t}")
                 for t in range(DHT)]
        wv_sb = persist.tile([P, CT * DH], BF16, tag="wv", name="wv")
        # x chunks: [128 in-ch, 8 kc x 512 n]
        xs = [persist.tile([P, CT * 512], BF16, tag=f"x{c}", name=f"x{c}")
              for c in range(NCH)]

        upool = ctx.enter_context(tc.tile_pool(name="upool", bufs=2,
                                               space="PSUM"))
        spool = ctx.enter_context(tc.tile_pool(name="spool", bufs=1,
                                               space="PSUM"))
        opool = ctx.enter_context(tc.tile_pool(name="opool", bufs=1,
                                               space="PSUM"))
        epool = ctx.enter_context(tc.tile_pool(name="epool", bufs=EPOOL))
        shufp = ctx.enter_context(tc.tile_pool(name="shufp", bufs=1))
        ttp = ctx.enter_context(tc.tile_pool(name="ttp", bufs=1))
        finp = ctx.enter_context(tc.tile_pool(name="finp", bufs=2))
        obp = ctx.enter_context(tc.tile_pool(name="obp", bufs=2))

        # ---------------- DMA (batched, first-need-first) ----------------
        def dma_x(c, kc0, kc1):
            dst = bass.AP(tensor=xs[c].tensor, offset=kc0 * 512,
                          ap=[list(xs[c].ap[0]), [512, kc1 - kc0], [1, 512]])
            src = bass.AP(tensor=xT.tensor, offset=kc0 * P * N + c * 512,
                          ap=[[N, P], [P * N, kc1 - kc0], [1, 512]])
            nc.sync.dma_start(dst, src)

        def dma_w(dst_tile, w_ap, t):
            dst = bass.AP(tensor=dst_tile.tensor, offset=0,
                          ap=[list(dst_tile.ap[0]), [P, CT], [1, P]])
            src = bass.AP(tensor=w_ap.tensor, offset=t * P,
                          ap=[[DH, P], [P * DH, CT], [1, P]])
            nc.sync.dma_start(dst, src)

        # first k-unit's tiles land first, in small pieces so matmuls can
        # start while the rest streams; x chunks 1-3 (big, ring-clogging)
        # go after everything the front needs
        dma_x(0, 0, 1)
        dma_w(wk_sb[0], wk, 0)
        dma_x(0, 1, 2)
        dma_x(0, 2, 4)
        dma_w(wq_sb[0], wq, 0)
        dma_x(0, 4, 8)
        dma_x(1, 0, 4)
        dma_x(1, 4, 8)
        nc.sync.dma_start(cos_c[0], cosp[:, 0:512])
        nc.sync.dma_start(sin_c[0], ssin[:, 0:512])
        wv_dst = bass.AP(tensor=wv_sb.tensor, offset=0,
                         ap=[list(wv_sb.ap[0]), [DH, CT], [1, DH]])
        wv_src = bass.AP(tensor=wv.tensor, offset=0,
                         ap=[[DH, P], [P * DH, CT], [1, DH]])
        nc.sync.dma_start(wv_dst, wv_src)
        dma_x(2, 0, 4)
        dma_x(2, 4, 8)
        nc.sync.dma_start(cos_c[1], cosp[:, 512:1024])
        nc.sync.dma_start(sin_c[1], ssin[:, 512:1024])
        dma_x(3, 0, 4)
        dma_x(3, 4, 8)
        for c in range(2, NCH):
            nc.sync.dma_start(cos_c[c], cosp[:, c * 512:(c + 1) * 512])
            nc.sync.dma_start(sin_c[c], ssin[:, c * 512:(c + 1) * 512])
        # ones columns into vS: offset 64 within each 72-stride head block
        ones_dst = bass.AP(tensor=vS.tensor, offset=64,
                           ap=[list(vS.ap[0]), [8 * VSTR, KT], [VSTR, 8]])
        ones_src = bass.AP(tensor=onesd.tensor, offset=0,
                           ap=[[P, P], [8, KT], [1, 8]])
        nc.sync.dma_start(ones_dst, ones_src)
        for t in range(1, DHT):
            dma_w(wk_sb[t], wk, t)
            dma_w(wq_sb[t], wq, t)
        for i in range(DHT):
            nc.sync.dma_start(wp_sb[i], wp[i * P:(i + 1) * P, :])

        # ---------------- emission-order tracking ----------------
        v_emitted = [False] * KT
        qk_emitted = set()   # ("q"|"k", t, c)

        # ---------------- unit step factories ----------------
        def qk_unit_steps(kind, w_sb, dst, t, c):
            hold = {}

            def step(kc):
                def f():
                    if kc == 0:
                        hold["ps"] = upool.tile([P, 512], F32, tag="ups", name="ups")
                    ps = hold["ps"]
                    nc.tensor.matmul(
                        ps, w_sb[t][:, kc * P:(kc + 1) * P],
                        xs[c][:, kc * 512:(kc + 1) * 512],
                        start=(kc == 0), stop=(kc == CT - 1))
                    if kc == CT - 1:
                        shuf = shufp.tile([P, 512], F32, tag="shuf", name="shuf")
                        nc.vector.stream_shuffle(shuf, ps, SHUF_MASK)
                        t1 = ttp.tile([P, 512], F32, tag="t1", name="t1")
                        nc.vector.tensor_mul(t1, ps, cos_c[c])
                        t2 = ttp.tile([P, 512], F32, tag="t2", name="t2")
                        nc.vector.tensor_mul(t2, shuf, sin_c[c])
                        nc.vector.tensor_add(dst[t][c], t1, t2)
                        qk_emitted.add((kind, t, c))
                return f
            return [step(kc) for kc in range(CT)]

        def v_unit_steps(c, nt4):
            i = c * 4 + nt4
            hold = {}

            def step(kc):
                def f():
                    if kc == 0:
                        hold["ps"] = upool.tile([P, 512], F32, tag="ups", name="ups")
                    ps = hold["ps"]
                    nc.tensor.matmul(
                        ps,
                        xs[c][:, kc * 512 + nt4 * P:kc * 512 + (nt4 + 1) * P],
                        wv_sb[:, kc * DH:(kc + 1) * DH],
                        start=(kc == 0), stop=(kc == CT - 1))
                    if kc == CT - 1:
                        v_view = bass.AP(
                            tensor=vS.tensor, offset=i * 8 * VSTR,
                            ap=[list(vS.ap[0]), [VSTR, 8], [1, 64]])
                        nc.vector.tensor_copy(
                            v_view, ps.rearrange("p (h d) -> p h d", h=8))
                        v_emitted[i] = True
                return f
            return [step(kc) for kc in range(CT)]

        # proj is split: hp0-2 partial sums run early (gated on the hp2
        # block of their chunk) and park in f32 views of the dead xs
        # memory; the hp3 final step adds partial + psum -> out.
        def pp_view(nt, fc):
            j = nt // 4
            slot = (j % 2) * 8 + (nt % 4) * 2 + fc
            base = xs[slot // 4][:, 0:CT * 512].bitcast(F32)
            return base[:, (slot % 4) * 512:(slot % 4 + 1) * 512]

        def proj_partial_steps(nt, fc):
            hold = {}

            def step(ct):
                def f():
                    if ct == 0:
                        hold["ps"] = upool.tile([P, 512], F32, tag="ups", name="ups")
                    ps = hold["ps"]
                    nc.tensor.matmul(
                        ps,
                        aoTc[ct][nt // 4][:, (nt % 4) * P:(nt % 4 + 1) * P],
                        wp_sb[ct][:, fc * 512:(fc + 1) * 512],
                        start=(ct == 0), stop=(ct == DHT - 2))
                    if ct == DHT - 2:
                        nc.vector.tensor_copy(pp_view(nt, fc), ps)
                return f
            return [step(ct) for ct in range(DHT - 1)]

        def proj_final_step(nt, fc):
            def f():
                ct = DHT - 1
                ps = upool.tile([P, 512], F32, tag="ups", name="ups")
                nc.tensor.matmul(
                    ps,
                    aoTc[ct][nt // 4][:, (nt % 4) * P:(nt % 4 + 1) * P],
                    wp_sb[ct][:, fc * 512:(fc + 1) * 512],
                    start=True, stop=True)
                ob = obp.tile([P, 512], BF16, tag="ob", name="ob")
                nc.vector.tensor_add(ob, pp_view(nt, fc), ps)
                nc.sync.dma_start(
                    out[nt * P:(nt + 1) * P, fc * 512:(fc + 1) * 512], ob)
            return [f]

        # ---------------- attention primitives ----------------
        pend = []          # FIFO of (blk, ki, e_tile)
        cur = {"blk": None, "o": None}
        finished = set()
        finished_at = {}
        gnow = [0]

        def finish_block(blk, o):
            hp, qc = blk // NCH, blk % NCH
            for par in range(2):
                pb = par * 64
                stage = finp.tile([P, 512], F32, tag="stg", name="stg")
                nc.vector.tensor_copy(stage[0:64, :], o[par][0:64, :])
                dd = finp.tile([P, 512], F32, tag="dd", name="dd")
                nc.vector.tensor_copy(dd[0:1, :], o[par][64:65, :])
                r = finp.tile([P, 512], F32, tag="r", name="r")
                nc.vector.reciprocal_approx_fast(r[0:1, :], dd[0:1, :])
                rb = finp.tile([P, 512], F32, tag="rb", name="rb")
                nc.gpsimd.partition_broadcast(
                    rb[0:64, :], r[0:1, :], channels=64)
                nc.vector.tensor_mul(
                    aoTc[hp][qc][pb:pb + 64, :], stage[0:64, :],
                    rb[0:64, :])
            finished.add(blk)
            finished_at[blk] = gnow[0]

        def pop_av():
            blk, ki, e = pend.pop(0)
            assert v_emitted[ki], f"v[{ki}] not emitted before AV pop"
            if blk != cur["blk"]:
                if cur["blk"] is not None:
                    finish_block(cur["blk"], cur["o"])
                cur["blk"] = blk
                cur["o"] = {par: opool.tile([P, 512], F32, tag=f"o{par}",
                                            name=f"o{par}")
                            for par in range(2)}
            for par in range(2):
                h = (blk // NCH) * 2 + par
                off = ki * 8 * VSTR + h * VSTR
                wv_view = bass.AP(tensor=vS.tensor, offset=off,
                                  ap=[list(vS.ap[0]), [1, 65]])
                nc.tensor.matmul(
                    cur["o"][par][0:65, :], wv_view, e[:, par],
                    start=(ki == 0), stop=(ki == KT - 1))
            return ki

        def attn_step(hp, qc, ki, g):
            assert ("k", hp, ki // 4) in qk_emitted, (hp, qc, ki, g)
            assert ("q", hp, qc) in qk_emitted, (hp, qc, ki, g)
            s_ps = spool.tile([P, 2, 512], F32, tag=f"s{g % 2}", name="s")
            for par in range(2):
                pb = par * 64
                nc.tensor.matmul(
                    s_ps[:, par],
                    kTc[hp][ki // 4][pb:pb + 64,
                                     (ki % 4) * P:(ki % 4 + 1) * P],
                    qTc[hp][qc][pb:pb + 64, :],
                    start=True, stop=True, tile_position=(pb, 0))
            e = epool.tile([P, 2, 512], BF16, tag="e", name="e")
            nc.scalar.activation(e, s_ps, Exp, scale=float(D) ** -0.5)
            pend.append((hp * NCH + qc, ki, e))

        def pops(g, cap=2):
            # hard: e-ring safety (AV(j) emitted before exp(j+EPOOL))
            while len(pend) > EPOOL - 2:
                pop_av()
            # soft: keep the backlog near the target lag
            done = 0
            while (pend and done < cap and len(pend) > _soft_lag(g)
                   and v_emitted[pend[0][1]]):
                ki = pop_av()
                done += 1
                if ki == KT - 1:
                    break  # boundary breather: let stage copies drain

        # ---------------- filler queue ----------------
        # each item: (gate_blk_or_None, step_thunk)
        fillers = []

        def push(steps, gate=None):
            for s in steps:
                fillers.append((gate, s))

        push(qk_unit_steps("k", wk_sb, kTc, 0, 1))
        push(qk_unit_steps("k", wk_sb, kTc, 0, 2))
        push(qk_unit_steps("k", wk_sb, kTc, 0, 3))
        push(qk_unit_steps("q", wq_sb, qTc, 0, 1))
        push(qk_unit_steps("q", wq_sb, qTc, 0, 2))
        for c in range(NCH):
            for nt4 in range(4):
                push(v_unit_steps(c, nt4))              # t0..t15
            if c == 2:
                push(qk_unit_steps("q", wq_sb, qTc, 0, 3))
        push(qk_unit_steps("k", wk_sb, kTc, 1, 0))
        push(qk_unit_steps("q", wq_sb, qTc, 1, 0))
        push(qk_unit_steps("k", wk_sb, kTc, 1, 1))
        push(qk_unit_steps("q", wq_sb, qTc, 1, 1))
        push(qk_unit_steps("k", wk_sb, kTc, 1, 2))
        push(qk_unit_steps("k", wk_sb, kTc, 1, 3))
        push(qk_unit_steps("q", wq_sb, qTc, 1, 2))
        push(qk_unit_steps("q", wq_sb, qTc, 1, 3))
        for t in (2, 3):
            for c in range(NCH):
                push(qk_unit_steps("k", wk_sb, kTc, t, c))
            for c in range(NCH):
                push(qk_unit_steps("q", wq_sb, qTc, t, c))
        # proj: partials pc_j gated on block (2,j); finals fc_j gated on
        # block (3,j).  Order [pc0 pc1 fc0 pc2 fc1 pc3 fc2] + fc3 at the
        # drain: every pp-slot-reusing write (pc2 reuses pc0 slots, pc3
        # reuses pc1 slots) is emitted after the final that reads them,
        # keeping the in-order DVE queue deadlock-free.
        def push_chunk_partials(j):
            for nt in range(j * 4, j * 4 + 4):
                for fc in range(2):
                    push(proj_partial_steps(nt, fc), gate=2 * NCH + j)

        def push_chunk_finals(j):
            for nt in range(j * 4, j * 4 + 4):
                for fc in range(2):
                    push(proj_final_step(nt, fc), gate=3 * NCH + j)

        push_chunk_partials(0)
        push_chunk_partials(1)
        push_chunk_finals(0)
        push_chunk_partials(2)
        push_chunk_finals(1)
        push_chunk_partials(3)
        push_chunk_finals(2)

        def emit_fillers(budget):
            n = 0
            while n < budget and fillers:
                gate, step = fillers[0]
                if gate is not None:
                    if gate not in finished:
                        break
                    # give the finish's DVE chain 2 ki of slack before the
                    # dependent proj matmuls hit the PE queue
                    if gnow[0] < finished_at.get(gate, 0) + 3:
                        break
                fillers.pop(0)
                step()
                n += 1
            return n

        # ---------------- head + main pacer ----------------
        for s in qk_unit_steps("k", wk_sb, kTc, 0, 0):
            s()
        for s in qk_unit_steps("q", wq_sb, qTc, 0, 0):
            s()

        for g in range(NBLK * KT):
            gnow[0] = g
            hp, qc, ki = g // 64, (g // 16) % 4, g % 16
            pops(g)
            attn_step(hp, qc, ki, g)
            emit_fillers(_budget(g))

        # drain: remaining AVs, gated projs, last finish, chunk-3 finals
        while pend:
            gnow[0] += 1
            pop_av()
            emit_fillers(2)
        gnow[0] += 1000
        # last block: latency-optimized finish (everything after it is
        # the serial dependency chain finish -> fc3 matmul -> add -> dma)
        o = cur["o"]
        rbs = []
        for par in range(2):
            dd = finp.tile([P, 512], F32, tag="dd", name="dd")
            nc.vector.tensor_copy(dd[0:1, :], o[par][64:65, :])
            r = finp.tile([P, 512], F32, tag="r", name="r")
            nc.vector.reciprocal_approx_fast(r[0:1, :], dd[0:1, :])
            rb = finp.tile([P, 512], F32, tag="rb", name="rb")
            nc.gpsimd.partition_broadcast(rb[0:64, :], r[0:1, :],
                                          channels=64)
            rbs.append(rb)
        for par in range(2):
            nc.vector.tensor_mul(
                aoTc[DHT - 1][NCH - 1][par * 64:par * 64 + 64, :],
                o[par][0:64, :], rbs[par][0:64, :])
        finished.add(NBLK - 1)
        emit_fillers(10 ** 9)
        assert not fillers, f"{len(fillers)} fillers never emitted"
        # chunk-3 finals: 8 matmuls into borrowed psum slots first (they
        # pipeline on the PE), then the adds + output DMAs
        slots = []
        for tag in ("s0", "s1"):
            sps = spool.tile([P, 2, 512], F32, tag=tag, name="s")
            slots += [sps[:, 0], sps[:, 1]]
        slots += [upool.tile([P, 512], F32, tag="ups", name="ups")
                  for _ in range(2)]
        slots += [opool.tile([P, 512], F32, tag=f"o{i}", name=f"o{i}")
                  for i in range(2)]
        ct = DHT - 1
        pairs = [(nt, fc) for nt in range((NCH - 1) * 4, NCH * 4)
                 for fc in range(2)]
        for (nt, fc), ps in zip(pairs, slots):
            nc.tensor.matmul(
                ps[0:128, :],
                aoTc[ct][nt // 4][:, (nt % 4) * P:(nt % 4 + 1) * P],
                wp_sb[ct][:, fc * 512:(fc + 1) * 512],
                start=True, stop=True)
        for (nt, fc), ps in zip(pairs, slots):
            ob = obp.tile([P, 512], BF16, tag="ob", name="ob")
            nc.vector.tensor_add(ob, pp_view(nt, fc), ps[0:128, :])
            nc.sync.dma_start(
                out[nt * P:(nt + 1) * P, fc * 512:(fc + 1) * 512], ob)


def build_nc():
    if "nc" in _CACHE:
        return _CACHE["nc"]
    import concourse.bass as bass
    import concourse.tile as tile
    from concourse import bacc, mybir

    nc = bacc.Bacc("TRN2", target_bir_lowering=False, debug=False,
                   enable_asserts=False, num_devices=NCORES)
    with tile.TileContext(nc) as tc:
        _emit(nc, tc, mybir, bass, tile)
    nc.compile()
    _CACHE["nc"] = nc
    return nc


def _perm128():
    """new_row -> old_row permutation within a 128-channel head pair."""
    perm = np.zeros(P, dtype=np.int64)
    sign = np.zeros(P, dtype=np.float32)
    dmap = np.zeros(P, dtype=np.int64)
    for new in range(P):
        Q, p = new // 32, new % 32
        h = Q // 2
        base = (Q % 2) * 16
        d = base + (32 if p >= 16 else 0) + (p % 16)
        perm[new] = h * 64 + d
        dmap[new] = d
        sign[new] = -1.0 if p < 16 else 1.0
    return perm, dmap, sign


def make_in_maps(x, rope_cos, rope_sin, w_qkv, w_proj):
    import ml_dtypes
    BF = ml_dtypes.bfloat16
    x = np.asarray(x, dtype=np.float32)
    rope_cos = np.asarray(rope_cos, dtype=np.float32)
    rope_sin = np.asarray(rope_sin, dtype=np.float32)
    w_qkv = np.asarray(w_qkv, dtype=np.float32)
    w_proj = np.asarray(w_proj, dtype=np.float32)

    perm, dmap, sign = _perm128()
    colperm = np.concatenate([t * P + perm for t in range(DHT)])

    cosp = np.ascontiguousarray(rope_cos.T[dmap, :]).astype(BF)   # [128, N]
    ssin = np.ascontiguousarray(
        rope_sin.T[dmap, :] * sign[:, None]).astype(BF)

    xTs = [np.ascontiguousarray(x[b].T).astype(BF) for b in range(B)]

    in_maps = []
    for core in range(NCORES):
        b = core // 2
        hg = core % 2
        wq_c = np.ascontiguousarray(
            w_qkv[hg * DH:(hg + 1) * DH, :].T[:, colperm]).astype(BF)
        wk_c = np.ascontiguousarray(
            w_qkv[C + hg * DH:C + (hg + 1) * DH, :].T[:, colperm]).astype(BF)
        in_maps.append({
            "xT": xTs[b],
            "wq": wq_c,
            "wk": wk_c,
            "wv": np.ascontiguousarray(
                w_qkv[2 * C + hg * DH:2 * C + (hg + 1) * DH, :].T).astype(BF),
            "wp": np.ascontiguousarray(
                w_proj[:, hg * DH:(hg + 1) * DH].T).astype(BF),
            "cosp": cosp,
            "ssin": ssin,
            "onesd": np.ones((P, P), dtype=BF),
        })
    return in_maps


def kernel(x, rope_cos, rope_sin, w_qkv, w_proj, b_proj, trace=False):
    from concourse.bass_utils import run_bass_kernel_spmd

    nc = build_nc()
    in_maps = make_in_maps(x, rope_cos, rope_sin, w_qkv, w_proj)
    res = run_bass_kernel_spmd(nc, in_maps, core_ids=list(range(NCORES)),
                               trace=trace)
    b_proj = np.asarray(b_proj, dtype=np.float32)
    final = np.empty((B, N, C), dtype=np.float32)
    for b in range(B):
        final[b] = (res.results[2 * b]["out"].astype(np.float32)
                    + res.results[2 * b + 1]["out"].astype(np.float32)
                    + b_proj)
    if trace:
        kernel.last_exec_time_ns = res.exec_time_ns
        kernel.last_results = res
    return final
